# revision 53
# baseline (speedup 1.0000x reference)
"""SSIM loss kernel for Trainium2, SPMD over 8 NeuronCores.

Inputs: img1, img2 [16,3,512,512] f32. Output: scalar mean SSIM (f32).
Sharding: batch dim 16 -> 2 per core (6 HxW planes/core); host sums the
per-core 512-lane partial sums.

Math (per pixel, 11x11 Gaussian window, C1=1e-4, C2=9e-4):
  streams x, y, w = x*y (2x band), s = x^2 + y^2 (PSUM-accumulated)
  ma = conv(x)+conv(y), mb = conv(x)-conv(y)     [pass-2 accum pairs]
  cd = 2*conv(xy), cs = conv(x^2)+conv(y^2)
  A = ma^2/2, B = mb^2/2; r2 = A-B, msq = A+B
  num = (cd - r2 + C2)(r2 + C1); den = (cs - msq + C2)(msq + C1)
  result = mean(num * recip_fast(den))

Implementation (engine budget is the constraint; DVE and ACT are the
co-bottlenecks at ~80%, PE ~70%, gpsimd deliberately IDLE - its ops
carry ~1-4us semaphore overhead and its SBUF port contends with DVE):
- Inputs staged fp16 host-side; hardware-DMA (sync + scalar HWDGE
  rings, one tensor each) into 5-window overlapped tiles.
- 11-tap separable conv as dense band matmuls: pass 1 convolves H
  data-stationary (output transposed so W lands on partitions; 5
  overlapping 128-row windows, stride 96); pass 2 convolves W
  band-stationary. fp16 taps ulp-nudged so sum(fl16(g)) == 1.
- Engine split: DVE = pre ops (w = x*y TT at 2x; s = x^2+y^2 fused
  custom) + r2/msq TTs + customs: nd (num), nd (den), and a fused
  recip-multiply (exponent-flip seed + one inline NR + numerator mul
  in one 1x pass; ripple +-1.7e-3, mean bias ~1e-4). ACT = ~19/plane
  pass-1 drains + the paired ma|mb Square (one [*,1024] op over a
  2-bank PSUM pair tile). s as a single stream also halves the
  s-stream pass-1 matmuls.
- Per-pixel SSIM summed on PE: acc_ps[1,512] += ones^T @ scr, a single
  PSUM accumulation group across the whole kernel; acc matmuls are
  deferred ~3 slots so they never stall the in-order PE queue.
- Software pipeline: pass1 slot t, pass2+post at t-3, acc at t-5;
  input tiles and yv triple-buffered (bufs=2 measurably stalls).
- PSUM budget (8 banks): pass1 rotation 3 + ma|mb pair 2 + cd 1 +
  cs 1 + acc 1.
"""

import math

import numpy as np

from concourse import bacc, bass, mybir, tile
from concourse.bass_utils import run_bass_kernel_spmd

B_FULL, C, H, W = 16, 3, 512, 512
N_CORES = 8
B_LOCAL = B_FULL // N_CORES          # 2
N_PLANES = B_LOCAL * C               # 6 spatial planes per core
KSZ = 11
PAD = KSZ // 2
SSIM_C1 = 0.01 ** 2
SSIM_C2 = 0.03 ** 2

STRIDE = 96
NWIN = 5
CHUNKS = [(0, 101), (101, 96), (197, 96), (293, 96), (389, 123)]
WF = NWIN * 512                      # 2560 free cols per plane

FP32 = mybir.dt.float32
FP16 = mybir.dt.float16

_OPS = {}


def _register_custom_ops():
    """Idempotently register the SSIM custom DVE ops."""
    global _OPS
    if _OPS:
        return _OPS
    import concourse.dve_ops as D
    from concourse.dve_spec import Spec, Src0, Src1, C0, C1, lower, _has_src1
    from concourse.dve_uop import DveOpSpec

    def reg(op):
        D.OPS.append(op)
        D._SUB_OPCODE_FOR_NAME[op.name] = D._CUSTOM_DVE_ROW_BASE + len(D.OPS) - 1
        D.CUSTOM_DVE_SPECS[op.name] = op.spec
        for ver in ("v3", "v4"):
            uops = lower(op.spec, ver=ver)
            so = DveOpSpec(name=op.name, opcode=D.get_dve_sub_opcode(op.name),
                           uops=uops, rd1_en=_has_src1(op.spec))
            op.uops_sha[ver] = so.sha(ver)
        return op

    from concourse.dve_spec import AluOp, Bin, sq

    def _ref_recip_mul(in0, in1, c0, c1, c2):
        not_x = (~in0.view(np.int32)).view(np.float32)
        y0 = not_x * np.float32(c0)
        y1 = y0 * (np.float32(c1) - in0 * y0)
        return y1 * in1.astype(np.float32)

    if "SSIM_ND_ANT" in D._SUB_OPCODE_FOR_NAME:
        nd = next(o for o in D.OPS if o.name == "SSIM_ND_ANT")
        rm = next(o for o in D.OPS if o.name == "SSIM_RECIP_MUL_ANT")
        ss = next(o for o in D.OPS if o.name == "SSIM_SQSUM_ANT")
    else:
        nd = reg(D.DveOp(
            "SSIM_ND_ANT",
            Spec(body=(Src0 - Src1 + C0) * (Src1 + C1),
                 reference=lambda in0, in1, s0, s1, imm2:
                     (in0.astype(np.float32) - in1 + s0)
                     * (in1.astype(np.float32) + s1)),
            subdim=False, uops_sha={}))
        # out = Src1 / Src0 (approx): exponent-flip seed + one inline NR,
        # then the numerator multiply in the same pass. Ripple +-1.7e-3,
        # mean bias ~1e-4 (the Chebyshev pair equioscillates post-NR).
        _nx = Bin(AluOp.BITWISE_NOT, Src0, Src0)
        _y0 = _nx * C0
        _y1 = _y0 * (C1 - Src0 * _y0)
        rm = reg(D.DveOp(
            "SSIM_RECIP_MUL_ANT",
            Spec(body=_y1 * Src1, reference=_ref_recip_mul),
            subdim=False, uops_sha={}))
        ss = reg(D.DveOp(
            "SSIM_SQSUM_ANT",
            Spec(body=sq(Src0) + sq(Src1),
                 reference=lambda in0, in1, s0, s1, imm2:
                     in0.astype(np.float32) ** 2 + in1.astype(np.float32) ** 2),
            subdim=False, uops_sha={}))
    # in1 is an interleaved (A,B) pair stream read as one 32-bit word per
    # position (step-2 fp16 AP): Src1 = lo = A, SRC_1_HI = hi = B. The ops
    # compute the r2/msq combination inline, removing the separate TTs.
    from concourse.dve_spec import InpSel, Leaf
    Src1Hi = Leaf(InpSel.SRC_1_HI)

    def _ref_nd_pair(sign):
        def _r(in0, in1, s0, s1, imm2):
            a = in1.reshape(in1.shape[0], -1)[:, 0::2].astype(np.float32)
            b = in1.reshape(in1.shape[0], -1)[:, 1::2].astype(np.float32)
            d = a + sign * b
            return (in0.reshape(d.shape).astype(np.float32) - d + s0) * (d + s1)
        return _r

    if "SSIM_NDD_ANT" in D._SUB_OPCODE_FOR_NAME:
        ndd = next(o for o in D.OPS if o.name == "SSIM_NDD_ANT")
        nds = next(o for o in D.OPS if o.name == "SSIM_NDS_ANT")
    else:
        _d = Src1 - Src1Hi
        ndd = reg(D.DveOp(
            "SSIM_NDD_ANT",
            Spec(body=(Src0 - _d + C0) * (_d + C1),
                 reference=_ref_nd_pair(-1.0)),
            subdim=False, uops_sha={}))
        _s = Src1 + Src1Hi
        nds = reg(D.DveOp(
            "SSIM_NDS_ANT",
            Spec(body=(Src0 - _s + C0) * (_s + C1),
                 reference=_ref_nd_pair(1.0)),
            subdim=False, uops_sha={}))
    _OPS = {"nd": nd, "rm": rm, "ss": ss, "ndd": ndd, "nds": nds,
            "recip_consts": D.RECIP_APPROX_FAST_CONSTS}
    return _OPS


def _gaussian_1d():
    x = np.arange(KSZ)
    g = np.exp(-((x - KSZ // 2) ** 2) / (2.0 * 1.5 ** 2))
    return (g / g.sum()).astype(np.float64)


def _gaussian_1d_f16():
    """fp16 taps nudged by +-1 ulp so sum(fl16(g)) == 1 to ~1e-7."""
    g16 = _gaussian_1d().astype(np.float16)
    for _ in range(200):
        e = g16.astype(np.float64).sum() - 1.0
        if abs(e) < 5e-8:
            break
        best = None
        for i in range(KSZ):
            step = np.nextafter(g16[i], np.float16(1.0 if e < 0 else 0.0))
            ne = e + (float(step) - float(g16[i]))
            if best is None or abs(ne) < abs(best[1]):
                best = (i, ne, step)
        i, ne, step = best
        if abs(ne) >= abs(e):
            break
        g16[i] = step
    return g16.astype(np.float64)


def _build_bands():
    """[128, 5*128] f16; window c at cols [128c, 128c+n_c).
    out[s+jj] = sum_r band[r, 128c+jj] * x[96c + r]."""
    g = _gaussian_1d_f16()
    bands = np.zeros((128, NWIN * 128), dtype=np.float64)
    for c, (s, n) in enumerate(CHUNKS):
        r0 = STRIDE * c
        for r in range(128):
            for jj in range(n):
                t = (r0 + r) - (s + jj) + PAD
                if 0 <= t < KSZ:
                    bands[r, c * 128 + jj] = g[t]
    return bands.astype(np.float16)


def _build_graph():
    ops = _register_custom_ops()
    nc = bacc.Bacc()
    img1 = nc.declare_dram_parameter("img1", [B_LOCAL, C, H, W], FP16, isOutput=False)
    img2 = nc.declare_dram_parameter("img2", [B_LOCAL, C, H, W], FP16, isOutput=False)
    bands = nc.declare_dram_parameter("bands", [128, NWIN * 128], FP16, isOutput=False)
    bandsn = nc.declare_dram_parameter("bandsn", [128, NWIN * 128], FP16, isOutput=False)
    bands2 = nc.declare_dram_parameter("bands2", [128, NWIN * 128], FP16, isOutput=False)
    out = nc.declare_dram_parameter("out", [1, 512], FP32, isOutput=True)

    Alu = mybir.AluOpType
    Act = mybir.ActivationFunctionType
    rc = ops["recip_consts"]
    INV_SQRT2 = 1.0 / math.sqrt(2.0)
    C12 = SSIM_C1 + SSIM_C2
    QS = {"x": 0, "y": 1, "w": 2, "s": 3}   # stream order in yv blocks
    N_ACC = N_PLANES * NWIN                 # total acc matmuls (30)

    with tile.TileContext(nc) as tc:
        with (
            tc.tile_pool(name="const_p", bufs=1) as const_p,
            tc.tile_pool(name="in16_p", bufs=3) as in16_p,
            tc.tile_pool(name="pre_p", bufs=2) as pre_p,
            tc.tile_pool(name="yv_p", bufs=3) as yv_p,
            tc.tile_pool(name="post_p", bufs=2) as post_p,
            tc.tile_pool(name="ps1_p", bufs=1, space="PSUM") as ps1_p,
            tc.tile_pool(name="ps2_p", bufs=1, space="PSUM") as ps2_p,
            tc.tile_pool(name="acc_p", bufs=1, space="PSUM") as acc_p,
        ):
            band_t = const_p.tile([128, NWIN * 128], FP16, name="band_t",
                                  tag="band_t")
            bandn_t = const_p.tile([128, NWIN * 128], FP16, name="bandn_t",
                                   tag="bandn_t")
            band2_t = const_p.tile([128, NWIN * 128], FP16, name="band2_t",
                                   tag="band2_t")
            nc.sync.dma_start(out=band_t[:], in_=bands[:, :])
            nc.sync.dma_start(out=bandn_t[:], in_=bandsn[:, :])
            nc.sync.dma_start(out=band2_t[:], in_=bands2[:, :])

            ones_t = const_p.tile([128, 1], FP16, name="ones_t", tag="ones_t")
            nc.vector.memset(ones_t[:], 1.0)

            acc_ps = acc_p.tile([1, 512], FP32, name="acc_ps")

            # --- fp16 input loads (hardware DMA), one per plane/tensor,
            # issued ~two planes ahead from the pipeline loop ---
            x16 = {}
            y16 = {}

            def emit_load(p, split=False):
                x16[p] = in16_p.tile([128, WF], FP16, name="x16", tag="x16")
                y16[p] = in16_p.tile([128, WF], FP16, name="y16", tag="y16")
                if not split:
                    for dst, src, eng in ((x16[p], img1, nc.sync),
                                          (y16[p], img2, nc.scalar)):
                        ap = bass.AP(src, p * H * W,
                                     [[W, 128], [STRIDE * W, NWIN], [1, W]])
                        eng.dma_start(
                            out=dst.rearrange("pt (c w) -> pt c w", c=NWIN),
                            in_=ap)
                    return
                # H-window-chunk loads ordered so windows 0-2 of BOTH
                # tensors land first (x on sync ring, y on scalar ring)
                for dst, src, eng in ((x16[p], img1, nc.sync),
                                      (y16[p], img2, nc.scalar)):
                    dv = dst.rearrange("pt (c w) -> pt c w", c=NWIN)
                    for c0, cn in ((0, 3), (3, 2)):
                        ap = bass.AP(src, p * H * W + c0 * STRIDE * W,
                                     [[W, 128], [STRIDE * W, cn], [1, W]])
                        eng.dma_start(out=dv[:, c0:c0 + cn, :], in_=ap)

            pre = {}     # plane -> dict of u/v/p/q stream tiles (fp16)

            def emit_pre(p, half=None):
                if half in (None, 0):
                    w16 = pre_p.tile([128, WF], FP16, name="w16", tag="w16")
                    s16 = pre_p.tile([128, WF], FP16, name="s16", tag="s16")
                    pre[p] = {"w": w16, "s": s16}
                w16 = pre[p]["w"]
                s16 = pre[p]["s"]
                if half is None:
                    sl = [slice(None)]
                else:
                    w0 = half * (W // 2)
                    sl3 = (slice(None), slice(None), slice(w0, w0 + W // 2))
                if p == 0:
                    for a, b in ((0, 1536), (1536, WF)):
                        nc.vector.tensor_tensor(
                            w16[:, a:b], x16[p][:, a:b], y16[p][:, a:b],
                            Alu.mult)
                        nc.vector._custom_dve(
                            ops["ss"], out=s16[:, a:b], in0=x16[p][:, a:b],
                            in1=y16[p][:, a:b])
                else:
                    nc.vector.tensor_tensor(w16[:], x16[p][:], y16[p][:],
                                            Alu.mult)
                    nc.vector._custom_dve(
                        ops["ss"], out=s16[:], in0=x16[p][:], in1=y16[p][:])

            yv = {}      # plane -> [128, 5*2048] f16 (per cw: u|v|p|q 512-blocks)

            def emit_pass1(p, cw):
                if cw == 0:
                    yv[p] = yv_p.tile([128, NWIN * 2048], FP16, name="yv",
                                      tag="yv")
                pr = pre[p]
                # stream-major: each stream's matmuls then its drain copy,
                # rotating over 3 PSUM banks.
                dve_drains = ("y",) if cw == 4 else ()
                srcs = {"x": (x16[p],), "y": (y16[p],), "w": (pr["w"],),
                        "s": (pr["s"],)}
                for qn in ("x", "y", "w", "s"):
                    p1 = ps1_p.tile([128, 512], FP32, name=f"p1{qn}", tag="p1",
                                    bufs=3)
                    bsrc = band2_t if qn == "w" else band_t
                    for c, (s, n) in enumerate(CHUNKS):
                        col = c * 512 + STRIDE * cw
                        bnd = bsrc[:, c * 128:c * 128 + n]
                        parts = srcs[qn]
                        for pi, ps in enumerate(parts):
                            nc.tensor.matmul(
                                p1[:, s:s + n], ps[:, col:col + 128],
                                bnd, start=(pi == 0),
                                stop=(pi == len(parts) - 1))
                    dst = yv[p][:, cw * 2048 + QS[qn] * 512:
                                cw * 2048 + QS[qn] * 512 + 512]
                    if qn in dve_drains:
                        nc.vector.tensor_copy(dst, p1[:, :])
                    else:
                        nc.scalar.copy(dst, p1[:, :])

            acc_n = [0]

            def emit_pass2_post(p, c2):
                s2, n2 = CHUNKS[c2]
                bnd = band_t[:, c2 * 128:c2 * 128 + n2]
                bndn = bandn_t[:, c2 * 128:c2 * 128 + n2]
                mamb = ps2_p.tile([128, 1024], FP32, name="mamb", tag="mamb")
                cdm = ps2_p.tile([128, 512], FP32, name="cdm", tag="cdm")
                csm = ps2_p.tile([128, 512], FP32, name="csm", tag="csm")
                yvx = yv[p][:, c2 * 2048 + QS["x"] * 512:c2 * 2048 + QS["x"] * 512 + 512]
                yvy = yv[p][:, c2 * 2048 + QS["y"] * 512:c2 * 2048 + QS["y"] * 512 + 512]
                yvw = yv[p][:, c2 * 2048 + QS["w"] * 512:c2 * 2048 + QS["w"] * 512 + 512]
                yvs = yv[p][:, c2 * 2048 + QS["s"] * 512:c2 * 2048 + QS["s"] * 512 + 512]
                nc.tensor.matmul(mamb[:n2, 0:512], bnd, yvx, start=True, stop=False)
                nc.tensor.matmul(mamb[:n2, 0:512], bnd, yvy, start=False, stop=True)
                nc.tensor.matmul(mamb[:n2, 512:1024], bnd, yvx, start=True, stop=False)
                nc.tensor.matmul(mamb[:n2, 512:1024], bndn, yvy, start=False, stop=True)
                nc.tensor.matmul(cdm[:n2, :], bnd, yvw, start=True, stop=True)
                nc.tensor.matmul(csm[:n2, :], bnd, yvs, start=True, stop=True)

                ABt = post_p.tile([128, 1024], FP16, name="ABt", tag="ABt")
                r2t = post_p.tile([128, 512], FP16, name="r2t", tag="r2t")
                msqt = post_p.tile([128, 512], FP16, name="msqt", tag="msqt")
                numt = post_p.tile([128, 512], FP16, name="numt", tag="numt")
                dent = post_p.tile([128, 512], FP32, name="dent", tag="dent")
                scr = post_p.tile([128, 512], FP16, name="scr", tag="scr",
                                  bufs=4)
                nc.scalar.activation(ABt[:n2, :], mamb[:n2, :], Act.Square,
                                     scale=INV_SQRT2)
                nc.vector.tensor_tensor(
                    r2t[:n2, :], ABt[:n2, 0:512], ABt[:n2, 512:1024],
                    Alu.subtract)
                nc.vector.tensor_tensor(
                    msqt[:n2, :], ABt[:n2, 0:512], ABt[:n2, 512:1024],
                    Alu.add)
                nc.vector._custom_dve(
                    ops["nd"], out=numt[:n2, :], in0=cdm[:n2, :],
                    in1=r2t[:n2, :], s0=SSIM_C2, s1=SSIM_C1)
                nc.vector._custom_dve(
                    ops["nd"], out=dent[:n2, :], in0=csm[:n2, :],
                    in1=msqt[:n2, :], s0=SSIM_C2, s1=SSIM_C1)
                nc.vector._custom_dve(
                    ops["rm"], out=scr[:n2, :], in0=dent[:n2, :],
                    in1=numt[:n2, :], s0=rc["s0"], s1=rc["s1"])
                acc_q.append((scr, n2))

            def emit_acc():
                scr, n2 = acc_q.popleft()
                k = acc_n[0]
                acc_n[0] += 1
                nc.tensor.matmul(acc_ps[:, :], ones_t[:n2, :], scr[:n2, :],
                                 start=(k == 0), stop=(k == N_ACC - 1))

            from collections import deque
            pending = deque()
            acc_q = deque()
            emit_load(0, split=True)
            emit_load(1)
            for p in range(N_PLANES):
                for cw in range(NWIN):
                    if p == 0 and cw == 0:
                        emit_pre(0)
                    emit_pass1(p, cw)
                    pending.append((p, cw))
                    if len(pending) > 2:
                        emit_pass2_post(*pending.popleft())
                    if len(acc_q) > 2:
                        emit_acc()
                    if cw == 1 and p + 2 < N_PLANES:
                        emit_load(p + 2)
                    if cw == 1 and p + 1 < N_PLANES:
                        emit_pre(p + 1)
            while pending:
                emit_pass2_post(*pending.popleft())
            while acc_q:
                emit_acc()

            out_sb = const_p.tile([1, 512], FP32, name="out_sb", tag="out_sb")
            nc.vector.tensor_copy(out_sb[:, :], acc_ps[:, :])
            nc.sync.dma_start(out=out[:, :], in_=out_sb[:, :])

    nc.compile()
    return nc


_NC_CACHE = None


def _in_maps(img1, img2):
    img1 = np.ascontiguousarray(img1, dtype=np.float32).astype(np.float16)
    img2 = np.ascontiguousarray(img2, dtype=np.float32).astype(np.float16)
    bands = _build_bands()
    return [
        {
            "img1": img1[i * B_LOCAL:(i + 1) * B_LOCAL],
            "img2": img2[i * B_LOCAL:(i + 1) * B_LOCAL],
            "bands": bands,
            "bandsn": (-bands.astype(np.float32)).astype(np.float16),
            "bands2": (bands.astype(np.float32) * 2.0).astype(np.float16),
        }
        for i in range(N_CORES)
    ]


def kernel(img1: np.ndarray, img2: np.ndarray) -> np.ndarray:
    global _NC_CACHE
    if _NC_CACHE is None:
        _NC_CACHE = _build_graph()
    nc = _NC_CACHE

    res = run_bass_kernel_spmd(nc, _in_maps(img1, img2), list(range(N_CORES)))
    total = np.float64(0.0)
    for r in res.results:
        total += np.asarray(r["out"], dtype=np.float64).sum()
    mean = total / (B_FULL * C * H * W)
    return np.array(mean, dtype=np.float32)
